# revision 1
# baseline (speedup 1.0000x reference)
"""Trainium2 Bass kernel for nn_Encoder (tri-modal Mamba encoder).

kernel(**inputs) takes FULL unsharded numpy inputs and returns the FULL
output (B, W, 2N+E, D). Batch B=8 is sharded across 8 NeuronCores (pure
data parallel, no collectives); params are replicated.

Device algorithm (per core, batch element b):
- Feature-major activations: (feature -> partitions, (seq, t) -> free),
  t innermost. All matmuls on PE in float32r.
- Work is split into 4 blocks of 128 sequences: node, trace[0:128],
  trace[128:256], log.
- Mamba selective scan: the (d_lo=8, s=16) expansion of dt/dtx is done by
  PE matmuls against host-built delta patterns; the A[d,s] multiply is
  folded into the ACT exp's per-partition scale; the recurrence
  h_t = dA*h + u runs on DVE tensor_tensor_scan along the free dim
  (sequence boundaries reset via dA[t=0]=0); sum over s is a PE matmul
  with an accumulating 0/1 pattern.
- LN in feature-major: mean/meansq via PE ones-matmul, rsqrt computed on a
  DMA-reshaped (128, *) view, scale factors replicated across partitions
  via K=1 ones-matmuls.
- Inter-phase tensors (mamba outputs, AddNorm inputs) spill to DRAM
  scratch to keep SBUF under budget.
"""

import ml_dtypes
import numpy as np
from contextlib import ExitStack

import concourse.bass as bass
import concourse.tile as tile
from concourse import bacc, mybir
from concourse.bass_utils import run_bass_kernel_spmd

D, DI, SS, KK, RR = 128, 256, 16, 4, 8
B, W, N, E = 8, 64, 128, 256
Q = 128                      # seqs per block
CBLK = Q * W                 # 8192 cols per block
CT = 512                     # column tile (8 seqs), 1 PSUM bank
f32 = mybir.dt.float32
f32r = mybir.dt.float32r
bf16 = mybir.dt.bfloat16
AF = mybir.ActivationFunctionType
OP = mybir.AluOpType

# blocks: (name, modality index, input tensor key, seq offset, out j offset)
BLOCKS = [("n", 0, "x_n", 0, 0),
          ("t0", 1, "x_t", 0, N),
          ("t1", 1, "x_t", Q, N + Q),
          ("l", 2, "x_l", 0, N + E)]
N_CORES = 8
LN_EPS = 1e-5
SCAN_BF16 = True   # bf16 scan datapath (dA/u/h/yh/Crep)


class Pack:
    """Column allocator for a (128, *) packed parameter array."""

    def __init__(self):
        self.cols = []
        self.off = {}
        self.n = 0

    def add(self, name, arr):
        arr = np.asarray(arr, np.float32)
        assert arr.ndim == 2 and arr.shape[0] <= 128
        a = np.zeros((128, arr.shape[1]), np.float32)
        a[: arr.shape[0]] = arr
        self.off[name] = (self.n, arr.shape[1])
        self.cols.append(a)
        self.n += arr.shape[1]

    def build(self):
        return np.concatenate(self.cols, axis=1)


def _host_pack(inp):
    bp = Pack()   # bf16 weights (sum patterns)
    for g in range(16):
        sm = np.zeros((128, 128), np.float32)
        for k in range(128):
            sm[k, g * 8 + k // 16] = 1.0
        bp.add(f"sum{g}", sm)
    wp = Pack()   # fp32 staged -> f32r on device (matmul lhsT / identities)
    vp = Pack()   # fp32 per-partition vectors (biases, taps, A-scales)

    for m in range(3):
        wp.add(f"win{m}", inp["mp_in"][m])                       # (D, 512)
        wxp = inp["mp_xproj"][m]                                 # (DI, 40)
        for cc in range(2):
            blk = wxp[cc * 128:(cc + 1) * 128]
            wp.add(f"wxpB{m}{cc}", np.tile(blk[:, RR:RR + SS], (1, 8)))
            wp.add(f"wxpC{m}{cc}", np.tile(blk[:, RR + SS:], (1, 8)))
            wp.add(f"wxpd{m}{cc}", blk[:, :RR])                  # (128, 8)
        dtw = inp["mp_dt_w"][m]                                  # (R, DI)
        for cc in range(2):
            wp.add(f"dtw{m}{cc}", dtw[:, cc * 128:(cc + 1) * 128])
        wout = inp["mp_out"][m]                                  # (DI, D)
        for cc in range(2):
            wp.add(f"wout{m}{cc}", wout[cc * 128:(cc + 1) * 128])
        wp.add(f"ff1{m}", inp["ff1_w"][m])                       # (D, 512)
        ff2 = inp["ff2_w"][m]                                    # (4D, D)
        for c4 in range(4):
            wp.add(f"ff2{m}{c4}", ff2[c4 * 128:(c4 + 1) * 128])
    mixw = inp["mix_w"]
    for kc in range(2):
        for mc in range(2):
            wp.add(f"mix{kc}{mc}", mixw[kc * 128:(kc + 1) * 128,
                                        mc * 128:(mc + 1) * 128])
    wp.add("onesD", np.full((128, 1), 1.0 / D, np.float32))
    wp.add("ones1", np.ones((1, 128), np.float32))
    for g in range(16):
        dl = np.zeros((128, 128), np.float32)
        for j in range(128):
            dl[g * 8 + j // 16, j] = 1.0
        wp.add(f"delta{g}", dl)

    vp.add("eps", np.full((128, 1), LN_EPS, np.float32))
    vp.add("I64", np.eye(64, dtype=np.float32))
    vp.add("I128", np.eye(128, dtype=np.float32))
    for m in range(3):
        cw = inp["mp_conv_w"][m]
        for cc in range(2):
            sl = slice(cc * 128, (cc + 1) * 128)
            vp.add(f"cw{m}{cc}", cw[sl])                          # 4 cols
            vp.add(f"cb{m}{cc}", inp["mp_conv_b"][m][sl, None])
            vp.add(f"dtb{m}{cc}", inp["mp_dt_b"][m][sl, None])
            vp.add(f"Dp{m}{cc}", inp["mp_D"][m][sl, None])
        for c4 in range(4):
            vp.add(f"f1b{m}{c4}", inp["ff1_b"][m][c4 * 128:(c4 + 1) * 128, None])
        vp.add(f"f2b{m}", inp["ff2_b"][m][:, None])
        vp.add(f"ang{m}", inp["an_g"][m][:, None])
        vp.add(f"anb{m}", inp["an_b"][m][:, None])
        vp.add(f"flg{m}", inp["fln_g"][m][:, None])
        vp.add(f"flb{m}", inp["fln_b"][m][:, None])
        A = -np.exp(np.asarray(inp["mp_Alog"][m], np.float64))    # (DI, S)
        for cc in range(2):
            for g in range(16):
                p = np.arange(128)
                col = A[cc * 128 + g * 8 + p // 16, p % 16]
                vp.add(f"Acol{m}{cc}{g}", col[:, None])
    for mc in range(2):
        vp.add(f"mixb{mc}", inp["mix_b"][mc * 128:(mc + 1) * 128, None])
    return wp, vp, bp


def _phase_a_block(tc, nc, aps, WR, VP, BR, I64, bname, mi, xkey, q_off,
                   ln_stats, statT, pools):
    """Mamba for one 128-seq block; result (y or y+x) -> DRAM scratch."""

    def mm(psum_ap, lhsT_ap, rhs_ap, start, stop, kp=128):
        nc.tensor.matmul(psum_ap, lhsT_ap[:kp, :], rhs_ap[:kp, :],
                         start=start, stop=stop)

    if True:
        bp, lp, mp, rp, pp = pools
        xT = bp.tile([128, CBLK], f32r, name=f"xT_{bname}", tag="xT")
        if True:
            for q0 in range(0, Q, 8):
                raw = lp.tile([64, 8 * 128], f32, name="raw", tag="raw")
                nc.sync.dma_start(raw[:],
                                  aps[xkey][:, q_off + q0:q_off + q0 + 8, :])
                pt = pp.tile([128, 8 * 64], f32, name="pt", tag="pP",
                             bufs=2)
                for i in range(8):
                    nc.tensor.transpose(pt[:, i * 64:(i + 1) * 64],
                                        raw[:, i * 128:(i + 1) * 128],
                                        I64[:64, :64])
                nc.vector.tensor_copy(xT[:, q0 * 64:(q0 + 8) * 64], pt[:])

        if True:
            for c0 in range(0, CBLK, CT):
                xt_t = xT[:, c0:c0 + CT]
                # ---- in_proj; conv/silu consume PSUM directly ----
                zs, xc = [], []
                for c in range(4):
                    pz = pp.tile([128, CT], f32, name="pz", tag="pz", bufs=2)
                    mm(pz[:], WR(f"win{mi}")[:, c * 128:(c + 1) * 128], xt_t,
                       True, True)
                    if c >= 2:
                        z = mp.tile([128, CT], f32, name=f"zs{c-2}",
                                    tag=f"zs{c-2}")
                        nc.scalar.activation(z[:], pz[:], AF.Silu)
                        zs.append(z)
                        continue
                    cc = c
                    acc = rp.tile([128, CT], f32, name="acc", tag="acc")
                    cw = VP(f"cw{mi}{cc}")
                    srcv = pz[:].rearrange("p (q t) -> p q t", t=W)
                    accv = acc[:].rearrange("p (q t) -> p q t", t=W)
                    nc.vector.tensor_scalar(acc[:], pz[:], cw[:, 3:4], None,
                                            OP.mult)
                    for k in range(3):
                        sh = 3 - k
                        nc.vector.scalar_tensor_tensor(
                            accv[:, :, sh:W], srcv[:, :, 0:W - sh],
                            cw[:, k:k + 1], accv[:, :, sh:W], OP.mult, OP.add)
                    xcc = mp.tile([128, CT], f32r, name=f"xc{cc}",
                                  tag=f"xc{cc}")
                    nc.scalar.activation(xcc[:], acc[:], AF.Silu,
                                         bias=VP(f"cb{mi}{cc}"))
                    xc.append(xcc)
                # ---- x_proj -> B_rep, C_rep, dt_in ----
                pB = pp.tile([128, CT], f32, name="pB", tag="pB", bufs=1)
                pC = pp.tile([128, CT], f32, name="pC", tag="pC", bufs=1)
                pdt = pp.tile([8, CT], f32, name="pdt", tag="pz", bufs=2)
                for cc in range(2):
                    mm(pB[:], WR(f"wxpB{mi}{cc}"), xc[cc][:], cc == 0, cc == 1)
                    mm(pC[:], WR(f"wxpC{mi}{cc}"), xc[cc][:], cc == 0, cc == 1)
                    mm(pdt[:], WR(f"wxpd{mi}{cc}"), xc[cc][:], cc == 0,
                       cc == 1)
                Brep = mp.tile([128, CT], f32, name="Brep", tag="Brep")
                nc.scalar.activation(Brep[:], pB[:], AF.Copy)
                Crep = mp.tile([128, CT], bf16 if SCAN_BF16 else f32, name="Crep", tag="Crep")
                nc.scalar.activation(Crep[:], pC[:], AF.Copy)
                dtin = mp.tile([8, CT], f32r, name="dtin", tag="dtin")
                nc.vector.tensor_copy(dtin[:], pdt[:])
                dts, dtx = [], []
                for cc in range(2):
                    pd = pp.tile([128, CT], f32, name="pd", tag="pz", bufs=2)
                    mm(pd[:], WR(f"dtw{mi}{cc}"), dtin[:], True, True, kp=8)
                    ez = rp.tile([128, CT], f32, name="ez", tag="ez")
                    nc.scalar.activation(ez[:], pd[:], AF.Exp,
                                         bias=VP(f"dtb{mi}{cc}"))
                    dt_c = mp.tile([128, CT], f32r, name=f"dt{cc}",
                                   tag=f"dt{cc}")
                    nc.scalar.activation(dt_c[:], ez[:], AF.Ln, bias=1.0)
                    dts.append(dt_c)
                    dx = mp.tile([128, CT], f32r, name=f"dtx{cc}",
                                 tag=f"dtx{cc}")
                    nc.gpsimd.tensor_mul(dx[:], dt_c[:], xc[cc][:])
                    dtx.append(dx)
                    # poison t=0 cols: exp(A*1e4) == 0 resets scan state
                    dtv = dt_c[:].rearrange("p (q t) -> p q t", t=W)
                    nc.vector.tensor_scalar(dtv[:, :, 0:1], dtv[:, :, 0:1],
                                            0.0, 1.0e4, OP.mult, OP.add)
                # ---- selective scan over (d_lo, s) groups ----
                # software-pipelined in waves of 8: front (PE mms, exp, u-mul)
                # then back (scan, yh, sum) so the DVE never waits on ACT.
                sdt = bf16 if SCAN_BF16 else f32
                gated = []
                for cc in range(2):
                    pY = pp.tile([128, CT], f32, name="pY", tag="pB", bufs=1)
                    for wave in (range(0, 8), range(8, 16)):
                        dAs, us = {}, {}
                        for g in wave:
                            pP = pp.tile([128, CT], f32, name="pP", tag="pP",
                                         bufs=2)
                            mm(pP[:], WR(f"delta{g}"), dts[cc][:], True, True)
                            dA = rp.tile([128, CT], sdt, name="dA", tag="dA",
                                         bufs=9)
                            nc.scalar.activation(dA[:], pP[:], AF.Exp,
                                                 scale=VP(f"Acol{mi}{cc}{g}"))
                            dAs[g] = dA
                            pX = pp.tile([128, CT], f32, name="pX", tag="pz",
                                         bufs=2)
                            mm(pX[:], WR(f"delta{g}"), dtx[cc][:], True, True)
                            u = rp.tile([128, CT], sdt, name="u", tag="u",
                                        bufs=9)
                            nc.vector.tensor_mul(u[:], pX[:], Brep[:])
                            us[g] = u
                        for g in wave:
                            h = rp.tile([128, CT], sdt, name="h", tag="h")
                            nc.vector.tensor_tensor_scan(h[:], dAs[g][:],
                                                         us[g][:], 0.0,
                                                         OP.mult, OP.add)
                            yh = rp.tile([128, CT], sdt, name="yh", tag="yh")
                            eng = nc.vector if g % 2 == 0 else nc.gpsimd
                            eng.tensor_mul(yh[:], h[:], Crep[:])
                            sumw = BR(f"sum{g}") if SCAN_BF16 else WR(f"sum{g}")
                            mm(pY[:], sumw, yh[:], g == 0, g == 15)
                    yg = rp.tile([128, CT], f32, name="yg", tag="yg")
                    nc.vector.scalar_tensor_tensor(yg[:], xc[cc][:],
                                                   VP(f"Dp{mi}{cc}"), pY[:],
                                                   OP.mult, OP.add)
                    gz = mp.tile([128, CT], f32r, name=f"gz{cc}",
                                 tag=f"gz{cc}")
                    nc.gpsimd.tensor_mul(gz[:], yg[:], zs[cc][:])
                    gated.append(gz)
                po = pp.tile([128, CT], f32, name="po", tag="pB", bufs=1)
                for cc in range(2):
                    mm(po[:], WR(f"wout{mi}{cc}"), gated[cc][:], cc == 0,
                       cc == 1)
                res = rp.tile([128, CT], f32r, name="res", tag="res")
                if bname in ("t0", "t1"):
                    nc.vector.tensor_add(res[:], po[:], xt_t)   # s = y + x
                    nc.sync.dma_start(aps[f"scr_s_{bname}"][:, c0:c0 + CT],
                                      res[:])
                    s2r = rp.tile([128, CT], f32r, name="s2r", tag="s2r")
                    nc.scalar.activation(s2r[:], res[:], AF.Square)
                    ln_stats(res[:], s2r[:], statT, c0, pp, rp)
                else:
                    nc.scalar.activation(res[:], po[:], AF.Copy)
                    nc.sync.dma_start(aps[f"scr_ym_{bname}"][:, c0:c0 + CT],
                                      res[:])


def _emit(ctx, tc, nc, aps, wp, vp, bpk):
    wpool = ctx.enter_context(tc.tile_pool(name="weights", bufs=1))
    wr = wpool.tile([128, wp.n], f32r, name="wr")
    vec = wpool.tile([128, vp.n], f32, name="vec")
    nc.sync.dma_start(vec[:], aps["vpack"][:])
    bw = wpool.tile([128, bpk.n], bf16, name="bw")
    nc.sync.dma_start(bw[:], aps["bpack"][:])
    with tc.tile_pool(name="wstage", bufs=1) as stpool:
        wstage = stpool.tile([128, wp.n], f32, name="wstage")
        nc.sync.dma_start(wstage[:], aps["wpack"][:])
        for o in range(0, wp.n, 8192):
            e = min(wp.n, o + 8192)
            nc.vector.tensor_copy(wr[:, o:e], wstage[:, o:e])

    def WR(name):
        o, c = wp.off[name]
        return wr[:, o:o + c]

    def VP(name):
        o, c = vp.off[name]
        return vec[:, o:o + c]

    def BR(name):
        o, c = bpk.off[name]
        return bw[:, o:o + c]

    def mm(psum_ap, lhsT_ap, rhs_ap, start, stop, kp=128):
        nc.tensor.matmul(psum_ap, lhsT_ap[:kp, :], rhs_ap[:kp, :],
                         start=start, stop=stop)

    I64 = VP("I64")
    I128 = VP("I128")

    def ln_stats(src_ap, s2_ap, statT, c0, ppool, spool, ptag="pP"):
        """Inline mean/meansq stats for a (128, CT) f32r tile into statT."""
        nw = CBLK // 128
        pm = ppool.tile([1, CT], f32, name="pm", tag=ptag, bufs=2)
        mm(pm[:], WR("onesD")[:, 0:1], src_ap, True, True)
        pq = ppool.tile([1, CT], f32, name="pq", tag=ptag, bufs=2)
        mm(pq[:], WR("onesD")[:, 0:1], s2_ap, True, True)
        p0 = c0 // nw
        smst = spool.tile([1, CT], f32, name="smst", tag="smst")
        nc.scalar.activation(smst[:], pm[:], AF.Copy)
        sqst = spool.tile([1, CT], f32, name="sqst", tag="sqst")
        nc.scalar.activation(sqst[:], pq[:], AF.Copy)
        nc.sync.dma_start(statT[p0:p0 + 8, 0:nw],
                          smst[:].rearrange("x (p w) -> x p w", w=nw))
        nc.sync.dma_start(statT[p0:p0 + 8, nw:2 * nw],
                          sqst[:].rearrange("x (p w) -> x p w", w=nw))

    statA = {}
    for bname, _, _, _, _ in BLOCKS:
        statA[bname] = wpool.tile([128, 2 * (CBLK // 128)], f32,
                                  name=f"statA_{bname}")

    # ---------------- phase A + B interleaved ----------------------
    # emit: A(n), A(l), mix, A(t0), A(t1) so mix overlaps trace mamba
    ABLOCKS = {b[0]: b for b in BLOCKS}
    with tc.tile_pool(name="a_b", bufs=1) as a_bp, \
         tc.tile_pool(name="a_ld", bufs=3) as a_lp, \
         tc.tile_pool(name="a_m", bufs=1) as a_mp, \
         tc.tile_pool(name="a_r", bufs=2) as a_rp, \
         tc.tile_pool(name="a_p", bufs=1, space="PSUM") as a_pp:
        pools = (a_bp, a_lp, a_mp, a_rp, a_pp)
        for bname in ("n", "l"):
            bn, mi, xkey, q_off, _ = ABLOCKS[bname]
            _phase_a_block(tc, nc, aps, WR, VP, BR, I64, bn, mi, xkey,
                           q_off, ln_stats, statA[bn], pools)
        with tc.tile_pool(name="mixw", bufs=2) as mxp, \
             tc.tile_pool(name="mixp", bufs=1, space="PSUM") as mxpp:
            for c0 in range(0, CBLK, CT):
                q0 = c0 // W
                cat = []
                for bname in ("n", "l"):
                    t_ = mxp.tile([128, CT], f32r, name=f"ym{bname}",
                                  tag=f"ym{bname}")
                    nc.sync.dma_start(t_[:],
                                      aps[f"scr_ym_{bname}"][:, c0:c0 + CT])
                    cat.append(t_)
                for mc, bname in enumerate(("n", "l")):
                    pmx = mxpp.tile([128, CT], f32, name="pmx", tag="pmx",
                                    bufs=2)
                    for kc in range(2):
                        mm(pmx[:], WR(f"mix{kc}{mc}"), cat[kc][:], kc == 0,
                           kc == 1)
                    ms = mxp.tile([128, CT], f32, name="ms", tag="ms")
                    nc.scalar.activation(ms[:], pmx[:], AF.Silu,
                                         bias=VP(f"mixb{mc}"))
                    # re-load + transpose x for the AddNorm residual
                    raw = mxp.tile([64, 8 * 128], f32, name="rawm", tag="rawm")
                    nc.sync.dma_start(raw[:], aps[f"x_{bname}"][:, q0:q0 + 8, :])
                    ptx = mxpp.tile([128, CT], f32, name="ptx", tag="pmx",
                                    bufs=2)
                    for i in range(8):
                        nc.tensor.transpose(ptx[:, i * 64:(i + 1) * 64],
                                            raw[:, i * 128:(i + 1) * 128],
                                            I64[:64, :64])
                    t2 = mxp.tile([128, CT], f32, name="t2", tag="t2")
                    nc.vector.tensor_add(t2[:], cat[mc][:], ms[:])
                    res = mxp.tile([128, CT], f32r, name="resm", tag="resm")
                    nc.vector.tensor_add(res[:], t2[:], ptx[:])
                    nc.sync.dma_start(aps[f"scr_s_{bname}"][:, c0:c0 + CT],
                                      res[:])
                    s2m = mxp.tile([128, CT], f32r, name="s2m", tag="s2m")
                    nc.scalar.activation(s2m[:], res[:], AF.Square)
                    ln_stats(res[:], s2m[:], statA[bname], c0, mxpp, mxp,
                             ptag="pmx")
        for bname in ("t0", "t1"):
            bn, mi, xkey, q_off, _ = ABLOCKS[bname]
            _phase_a_block(tc, nc, aps, WR, VP, BR, I64, bn, mi, xkey,
                           q_off, ln_stats, statA[bn], pools)

    # ---------------- phase C: LN apply -> FFN -> LN -> out --------
    def ln_finish(tag, statT, pool):
        """statT (m|q) -> (r_t, nmr) f32r tiles, p-major chunk layout."""
        nw = CBLK // 128
        m_t, q_t = statT[:, 0:nw], statT[:, nw:2 * nw]
        var = pool.tile([128, nw], f32, name=f"var_{tag}", tag="lnvar")
        nc.vector.tensor_mul(var[:], m_t, m_t)
        nc.vector.tensor_sub(var[:], q_t, var[:])
        sd = pool.tile([128, nw], f32, name=f"sd_{tag}", tag="lnsd")
        nc.scalar.activation(sd[:], var[:], AF.Sqrt, bias=VP("eps"))
        r_t = pool.tile([128, nw], f32r, name=f"r_{tag}", tag="lnr")
        with nc.allow_low_precision(reason="f32r LN scale factors"):
            nc.vector.reciprocal(r_t[:], sd[:])
        nmr = pool.tile([128, nw], f32r, name=f"nmr_{tag}", tag="lnnmr")
        nc.vector.tensor_mul(nmr[:], m_t, r_t[:])
        nc.vector.tensor_scalar(nmr[:], nmr[:], -1.0, None, OP.mult)
        return r_t, nmr

    def ln_apply(src_ap, r_t, nmr, c0, g_ap, b_ap, out_ap, pool, ppool):
        nw = CBLK // 128
        p0 = c0 // nw
        rst = pool.tile([1, CT], f32r, name="rst", tag="rst")
        nc.sync.dma_start(rst[:].rearrange("x (p w) -> x p w", w=nw),
                          r_t[p0:p0 + 8, :])
        nmst = pool.tile([1, CT], f32r, name="nmst", tag="nmst")
        nc.sync.dma_start(nmst[:].rearrange("x (p w) -> x p w", w=nw),
                          nmr[p0:p0 + 8, :])
        prep = ppool.tile([128, CT], f32, name="prep", tag="prep", bufs=1)
        mm(prep[:], WR("ones1"), rst[:], True, True, kp=1)
        pnm = ppool.tile([128, CT], f32, name="pnm", tag="pnm", bufs=1)
        mm(pnm[:], WR("ones1"), nmst[:], True, True, kp=1)
        t1 = pool.tile([128, CT], f32, name="t1", tag="t1")
        nc.vector.tensor_mul(t1[:], src_ap, prep[:])
        nc.vector.tensor_add(t1[:], t1[:], pnm[:])
        nc.vector.tensor_scalar(out_ap, t1[:], g_ap, b_ap, OP.mult, OP.add)

    with tc.tile_pool(name="c_per", bufs=1) as cper, \
         tc.tile_pool(name="c_w", bufs=2) as cp, \
         tc.tile_pool(name="c_p", bufs=2, space="PSUM") as cpp:
        for bname, mi, _, _, j0 in BLOCKS:
            rA, nmA = ln_finish(f"a{bname}", statA[bname], cp)
            statF = cper.tile([128, 2 * (CBLK // 128)], f32,
                              name=f"statF_{bname}", tag="statF", bufs=2)
            n1 = cper.tile([128, CBLK], f32r, name=f"n1_{bname}",
                           tag="big", bufs=3)
            sf = cper.tile([128, CBLK], f32r, name=f"sf_{bname}",
                           tag="big", bufs=3)
            for c0 in range(0, CBLK, CT):
                sld = cp.tile([128, CT], f32r, name="sld", tag="sld")
                nc.sync.dma_start(sld[:], aps[f"scr_s_{bname}"][:, c0:c0 + CT])
                ln_apply(sld[:], rA, nmA, c0, VP(f"ang{mi}"), VP(f"anb{mi}"),
                         n1[:, c0:c0 + CT], cp, cpp)
                hh = []
                for c4 in range(4):
                    pf = cpp.tile([128, CT], f32, name="pf", tag="pf", bufs=2)
                    mm(pf[:], WR(f"ff1{mi}")[:, c4 * 128:(c4 + 1) * 128],
                       n1[:, c0:c0 + CT], True, True)
                    hc = cp.tile([128, CT], f32r, name=f"hh{c4}", tag=f"hh{c4}")
                    nc.scalar.activation(hc[:], pf[:], AF.Lrelu,
                                         bias=VP(f"f1b{mi}{c4}"), alpha=0.01)
                    hh.append(hc)
                pf2 = cpp.tile([128, CT], f32, name="pf2", tag="pf2", bufs=1)
                for c4 in range(4):
                    mm(pf2[:], WR(f"ff2{mi}{c4}"), hh[c4][:], c4 == 0, c4 == 3)
                nc.vector.scalar_tensor_tensor(sf[:, c0:c0 + CT], pf2[:],
                                               VP(f"f2b{mi}"),
                                               n1[:, c0:c0 + CT],
                                               OP.add, OP.add)
                s2f = cp.tile([128, CT], f32r, name="s2f", tag="s2f")
                nc.scalar.activation(s2f[:], sf[:, c0:c0 + CT], AF.Square)
                ln_stats(sf[:, c0:c0 + CT], s2f[:], statF, c0, cpp, cp)
            rF, nmF = ln_finish(f"f{bname}", statF, cp)
            n2 = cper.tile([128, CBLK], f32, name=f"n2_{bname}",
                           tag="big", bufs=3)
            for c0 in range(0, CBLK, CT):
                ln_apply(sf[:, c0:c0 + CT], rF, nmF, c0, VP(f"flg{mi}"),
                         VP(f"flb{mi}"), n2[:, c0:c0 + CT], cp, cpp)
            for q0 in range(0, Q, 2):
                pt = cpp.tile([128, 128], f32, name="pto", tag="pto", bufs=1)
                nc.tensor.transpose(pt[:], n2[:, q0 * 64:(q0 + 2) * 64], I128)
                ot = cp.tile([128, 128], f32, name="ot", tag="ot")
                nc.vector.tensor_copy(ot[:], pt[:])
                nc.sync.dma_start(
                    aps["out"][:, j0 + q0:j0 + q0 + 2, :]
                    .rearrange("t q d -> q t d"), ot[:])


def _build_program(wp, vp, bpk):
    nc = bacc.Bacc("TRN2", target_bir_lowering=False, debug=False,
                   num_devices=N_CORES)
    aps = {}
    aps["x_n"] = nc.dram_tensor("x_n", [W, N, D], f32, kind="ExternalInput").ap()
    aps["x_t"] = nc.dram_tensor("x_t", [W, E, D], f32, kind="ExternalInput").ap()
    aps["x_l"] = nc.dram_tensor("x_l", [W, N, D], f32, kind="ExternalInput").ap()
    aps["wpack"] = nc.dram_tensor("wpack", [128, wp.n], f32,
                                  kind="ExternalInput").ap()
    aps["vpack"] = nc.dram_tensor("vpack", [128, vp.n], f32,
                                  kind="ExternalInput").ap()
    aps["bpack"] = nc.dram_tensor("bpack", [128, bpk.n], bf16,
                                  kind="ExternalInput").ap()
    aps["out"] = nc.dram_tensor("out", [W, 2 * N + E, D], f32,
                                kind="ExternalOutput").ap()
    for bname, _, _, _, _ in BLOCKS:
        if bname in ("t0", "t1"):
            aps[f"scr_s_{bname}"] = nc.dram_tensor(
                f"scr_s_{bname}", [128, CBLK], f32r).ap()
        else:
            aps[f"scr_ym_{bname}"] = nc.dram_tensor(
                f"scr_ym_{bname}", [128, CBLK], f32r).ap()
            aps[f"scr_s_{bname}"] = nc.dram_tensor(
                f"scr_s_{bname}", [128, CBLK], f32r).ap()

    with tile.TileContext(nc) as tc:
        with ExitStack() as ctx:
            _emit(ctx, tc, nc, aps, wp, vp, bpk)
    nc.compile()
    return nc


_CACHE = {}


def kernel(**inputs):
    wp, vp, bpk = _host_pack(inputs)
    if "prog" not in _CACHE:
        _CACHE["prog"] = _build_program(wp, vp, bpk)
    nc = _CACHE["prog"]
    wpack, vpack = wp.build(), vp.build()
    bpack = bpk.build().astype(ml_dtypes.bfloat16)
    in_maps = []
    for b in range(B):
        in_maps.append({
            "x_n": np.ascontiguousarray(inputs["x_node"][b]),
            "x_t": np.ascontiguousarray(inputs["x_trace"][b]),
            "x_l": np.ascontiguousarray(inputs["x_log"][b]),
            "wpack": wpack,
            "vpack": vpack,
            "bpack": bpack,
        })
    res = run_bass_kernel_spmd(nc, in_maps, list(range(N_CORES)))
    out = np.stack([res.results[b]["out"] for b in range(B)], axis=0)
    return out.astype(np.float32)

